# revision 30
# baseline (speedup 1.0000x reference)
"""CRF loss (sum reduction) on 8 Trainium2 NeuronCores.

Strategy (data-parallel, batch sharded 8 ways, B_local=64 per core):
  * Denominator (log-partition): linear-space scaled forward algorithm.
    state[k,col]; step: state = (M^T state) * E_t with M = exp(transitions)
    as the stationary matmul lhsT and E_t = exp(em_t + bias - C0) computed
    ON HOST and streamed as bf16 (halves HBM traffic vs f32 em and removes
    the on-device exp pass entirely).
  * The serial T=512 scan is split into parallel-in-time segments, each
    warm-started one step early from a uniform vector (the transition
    matrix is a strong Hilbert-metric contraction, ~1e-2/step measured, so
    W=1 warmup leaves ~2e-4 nats of seam error). Segments are grouped into
    5 independent chains sized to balance engine load:
      - 2 "direct" chains (512 cols, 8 segs x 10 steps): DVE
        scalar_tensor_tensor straight from f32 PSUM (1x mode).
      - 2 "pair" chains (1024 cols, 16 segs x 9 steps): ScalarE evicts the
        two PSUM banks as one [K,1024] activation-copy to bf16 SBUF, then
        DVE multiplies all-bf16 at 2x mode.
      - 1 "solo" evict chain (512 cols, 8 segs x 8 steps).
    7 PSUM banks for the scan + 1 bank for events = 8.
  * No renormalization: per-column magnitudes stay O(1) by the -C0 bias;
    column sums are measured (ones-vector matmul -> one PSUM partition
    row) after the warmup row and after the last row; host takes logs.
  * Numerator (path score of the given tags) is exact and tiny
    (O(T*B) gathers): computed on host in f64.
"""

import sys
import numpy as np

for _p in ("/opt/trn_rl_repo",):
    if _p not in sys.path:
        sys.path.insert(0, _p)

import ml_dtypes

BF16 = ml_dtypes.bfloat16

T, B, K = 512, 512, 128
NCORES = 8
BL = B // NCORES            # 64 batch per core
C0 = 5.354                  # per-step log-scale compensation
W = 1                       # warmup rows per segment

# chain configs: (width_cols, TSEG, t0).  nseg = width // BL.
# kinds: direct = DVE STT straight from f32 PSUM (1x);
#        pair   = ScalarE [K,1024] eviction -> DVE bf16 TT (2x);
#        solo   = ScalarE [K,512] eviction -> DVE bf16 TT.
# coverage: 8*10 + 8*10 + 16*9 + 16*9 + 8*8 = 512 steps.
CHAINS = [
    dict(w=512,  tseg=10, t0=0,   kind="direct"),
    dict(w=512,  tseg=10, t0=80,  kind="direct"),
    dict(w=1024, tseg=9,  t0=160, kind="pair"),
    dict(w=1024, tseg=9,  t0=304, kind="pair"),
    dict(w=512,  tseg=8,  t0=448, kind="solo"),
]
for _c in CHAINS:
    _c["nseg"] = _c["w"] // BL
    _c["rows"] = _c["tseg"] + W
NROUND = max(c["rows"] for c in CHAINS)

# stream layout: round-major; within a round, chains in index order.
_CHUNK_START = []   # global col where round r starts
_SLOT_OFF = []      # per round: {chain: offset within round}
_ROUND_W = []
_off = 0
for _r in range(NROUND):
    _CHUNK_START.append(_off)
    offs = {}
    rw = 0
    for _ci, _c in enumerate(CHAINS):
        if _r < _c["rows"]:
            offs[_ci] = rw
            rw += _c["w"]
    _SLOT_OFF.append(offs)
    _ROUND_W.append(rw)
    _off += rw
NSTREAM = _off   # 36352

# final-event blocks: one per 512-col block of each chain; each block emits
# 4 transposed colsum matmuls ([K,128] stationary x ones -> [128,1]).
_EV_BLOCKS = []  # (chain_idx, block_idx)
for _ci, _c in enumerate(CHAINS):
    for _k in range(_c["w"] // 512):
        _EV_BLOCKS.append((_ci, _k))
NBLK = len(_EV_BLOCKS)          # 7
NEVCOLS = 4 * NBLK              # 28 psum cols, one per 128-state-col group


def _build_program():
    import concourse.bass as bass
    import concourse.tile as tile
    from concourse import mybir
    from contextlib import ExitStack

    # --- patch: walrus here rejects >1 sync-wait on the Tile final Drain ---
    from concourse.tile import ScopedClock

    def _patched_drain_and_barrier(self, tick_clock, wait_clock):
        nc = self.nc
        drain_inst = nc.sync.drain()
        wait_clock.add_sem_waits(
            drain_inst.ins, ScopedClock({None: tick_clock.global_clock})
        )
        si = drain_inst.ins.sync_info
        if si is not None and si.on_wait and len(si.on_wait) > 1:
            extra = list(si.on_wait[1:])
            del si.on_wait[1:]
            for w in extra:
                nop = nc.sync.nop()
                nop.ins.sync_info = mybir.SyncInfo(on_wait=[w], on_update=[])
        nc.all_engine_barrier()
        assert self.sems is not None
        popped = nc._tile_sem_poison_stack.pop()
        assert popped is self._sem_poison
        nc.clear_and_free_semaphores(list(self.sems.allocated().values()))
        nc.all_engine_barrier()

    tile.TileContext._drain_and_barrier = _patched_drain_and_barrier

    # --- patch 2: same walrus cap applies to every instruction type; spill
    # extra waits onto same-engine NOPs inserted just before. ---
    import bass_rust

    def _spill_excess_waits(nc_, cap=1):
        ctr = 0
        for f in nc_.m.functions:
            for bb in f.blocks:
                newlist = []
                for inst in bb.instructions:
                    si = getattr(inst, "sync_info", None)
                    if si is not None and si.on_wait and len(si.on_wait) > cap:
                        extra = list(si.on_wait[cap:])
                        del si.on_wait[cap:]
                        for w in extra:
                            ctr += 1
                            nop = bass_rust.InstNoOp(name=f"I-waitfix-{ctr}")
                            nop.engine = inst.engine
                            nop.sync_info = mybir.SyncInfo(on_wait=[w], on_update=[])
                            newlist.append(nop)
                    newlist.append(inst)
                bb.instructions[:] = newlist

    f32 = mybir.dt.float32
    bf16 = mybir.dt.bfloat16
    AF = mybir.ActivationFunctionType
    OP = mybir.AluOpType

    nc = bass.Bass()
    emt = nc.declare_dram_parameter("emt", [K, NSTREAM], bf16, isOutput=False)
    mexp = nc.declare_dram_parameter("mexp", [K, K], bf16, isOutput=False)
    ev_out = nc.declare_dram_parameter("ev", [K, NEVCOLS], f32, isOutput=True)

    with ExitStack() as ctx:
        tc = ctx.enter_context(tile.TileContext(nc))
        singles = ctx.enter_context(tc.tile_pool(name="singles", bufs=1))
        psum = ctx.enter_context(tc.tile_pool(name="psum", bufs=1, space="PSUM"))

        def tt_mult(out, in0, in1, eng=None):
            # true InstTensorTensor: hits DVE 2x_1p mode for all-bf16 SBUF
            # operands (scalar_tensor_tensor measures 1x on HW)
            v = eng if eng is not None else nc.vector
            return v.add_instruction(
                mybir.InstTensorTensor(
                    name=v.bass.get_next_instruction_name(),
                    op=OP.mult,
                    ins=[v.lower_ap(in0), v.lower_ap(in1)],
                    outs=[v.lower_ap(out)],
                )
            )

        # constants
        mexp_sb = singles.tile([K, K], bf16)
        nc.sync.dma_start(out=mexp_sb[:], in_=mexp[:])
        ones_k = singles.tile([K, 1], bf16)
        nc.vector.memset(ones_k[:], 1.0)

        # streamed E chunks: fine-grained early (fast scan start), coarse
        # later (per-chunk DMA bandwidth rises with size); alternate between
        # the two HWDGE rings (SP via nc.sync, ACT via nc.scalar) so
        # transfers overlap across rings.
        groups = [[0], [1], [2], [3, 4], [5, 6], [7, 8], [9, 10]]
        Gtiles = []
        group_of_round = {}
        for gi, rounds in enumerate(groups):
            gw = sum(_ROUND_W[r] for r in rounds)
            gt = singles.tile([K, gw], bf16, name=f"Eg{gi}", tag=f"Eg{gi}")
            Gtiles.append(gt)
            for r in rounds:
                group_of_round[r] = gi
            lo = _CHUNK_START[rounds[0]]
            eng = nc.sync if gi % 2 == 0 else nc.scalar
            eng.dma_start(out=gt[:], in_=emt[:, lo : lo + gw])

        def esl(r, ci, off=0, width=None):
            gi = group_of_round[r]
            base = _CHUNK_START[r] - _CHUNK_START[groups[gi][0]]
            lo = base + _SLOT_OFF[r][ci] + off
            w = CHAINS[ci]["w"] if width is None else width
            return Gtiles[gi][:, lo : lo + w]

        # states + evict buffers
        st = []
        ev_sb = []
        ps = []
        for ci, c in enumerate(CHAINS):
            s = singles.tile([K, c["w"]], bf16, name=f"st{ci}", tag=f"st{ci}")
            st.append(s)
            if c["kind"] != "direct":
                evb = singles.tile([K, c["w"]], bf16, name=f"evb{ci}", tag=f"evb{ci}")
                ev_sb.append(evb)
            else:
                ev_sb.append(None)
            psc = psum.tile([K, c["w"]], f32, name=f"ps{ci}", tag=f"ps{ci}")
            ps.append(psc)

        evt = psum.tile([K, NEVCOLS], f32, name="evt", tag="evt")
        evt_sb = singles.tile([K, NEVCOLS], f32, name="evt_sb")

        def emit_colsums(blocks):
            # transposed colsum: out[m,0] = sum_p st[p, base+m]
            for p, ci, k in blocks:
                for h in range(4):
                    base = 512 * k + 128 * h
                    nc.tensor.matmul(
                        evt[:, 4 * p + h : 4 * p + h + 1],
                        st[ci][:, base : base + 128],
                        ones_k[:],
                        start=True,
                        stop=True,
                    )

        # ---- the scan ----
        # row 0 is data-only: each segment warm-starts from E(t_w) directly
        # (M^T x ~ 1*colsum(x), so E(t_w) is already a one-step-warmed
        # direction); row 1's matmul reads the row-0 E slice as rhs.
        for r in range(1, NROUND):
            # longest dependency path first: pool, then pair, then direct
            prio = {"pool": 0, "pair": 1, "solo": 1, "direct": 2}
            order = sorted(range(len(CHAINS)), key=lambda ci: prio[CHAINS[ci]["kind"]])
            for ci in order:
                c = CHAINS[ci]
                if r >= c["rows"]:
                    continue
                Esl = esl(r, ci)
                # matmuls (one per 512-col bank); row 1 reads row-0 E as rhs
                for k in range(c["w"] // 512):
                    rhs = (
                        esl(0, ci, off=512 * k, width=512)
                        if r == 1
                        else st[ci][:, 512 * k : 512 * k + 512]
                    )
                    nc.tensor.matmul(
                        ps[ci][:, 512 * k : 512 * k + 512],
                        mexp_sb[:],
                        rhs,
                        start=True,
                        stop=True,
                    )
                if c["kind"] == "direct":
                    nc.vector.scalar_tensor_tensor(
                        out=st[ci][:],
                        in0=ps[ci][:],
                        scalar=1.0,
                        in1=Esl,
                        op0=OP.mult,
                        op1=OP.mult,
                    )
                else:
                    nc.scalar.activation(
                        ev_sb[ci][:], ps[ci][:], AF.Copy, bias=0.0, scale=1.0
                    )
                    tt_mult(st[ci][:], ev_sb[ci][:], Esl)
            if r == W:
                # exact reset of global segment 0 (chain 0, seg 0) to
                # a_0 = E_{t=0} (start bias folded in on host)
                nc.vector.tensor_copy(st[0][:, 0:BL], esl(W, 0, width=BL))
            # final colsum events for chains ending this round
            ending = [
                (p, ci, k)
                for p, (ci, k) in enumerate(_EV_BLOCKS)
                if CHAINS[ci]["rows"] == r + 1
            ]
            if ending:
                emit_colsums(ending)

        nc.vector.tensor_copy(evt_sb[:], evt[:])
        nc.sync.dma_start(out=ev_out[:], in_=evt_sb[:])

    _spill_excess_waits(nc)
    return nc


def _host_prep(emissions, start_transitions, end_transitions, transitions):
    """Per-core emt stream: E = exp(em + bias - C0) in bf16, round-major.

    Also returns the warm-start column sums (state after warmup row 0 is
    deterministically (M^T 1) * E, so ln of its colsum is host-computable).
    """
    in_maps = []
    warms = []
    biast = np.zeros((T, K), np.float32)
    biast[0] += start_transitions
    biast[-1] += end_transitions
    for c in range(NCORES):
        b0 = c * BL
        em_t = np.ascontiguousarray(
            emissions[:, b0 : b0 + BL, :].transpose(0, 2, 1)
        )  # [T, K, BL]
        Ebig = np.exp(em_t - C0 + biast[:, :, None]).astype(BF16)  # [T, K, BL]
        emt = np.empty((K, NSTREAM), BF16)
        for r in range(NROUND):
            for ci, ch in enumerate(CHAINS):
                if r >= ch["rows"]:
                    continue
                t_arr = np.clip(
                    ch["t0"] + np.arange(ch["nseg"]) * ch["tseg"] + (r - W), 0, T - 1
                )
                blk = Ebig[t_arr]  # [nseg, K, BL]
                lo = _CHUNK_START[r] + _SLOT_OFF[r][ci]
                emt[:, lo : lo + ch["w"]] = blk.transpose(1, 0, 2).reshape(
                    K, ch["w"]
                )
        in_maps.append({"emt": emt})
        wc = []
        for ch in CHAINS:
            t_w = np.clip(
                ch["t0"] + np.arange(ch["nseg"]) * ch["tseg"] - 1, 0, T - 1
            )
            # warm start is E(t_w) itself: warm colsum[j, b] = sum_k E
            wc.append(Ebig[t_w].astype(np.float64).sum(axis=1))
        warms.append(wc)
    return in_maps, warms


def _host_numerator(emissions, tags, mask, start_transitions, end_transitions,
                    transitions):
    em = emissions.astype(np.float64)
    maskf = mask.astype(np.float64)
    b_idx = np.arange(em.shape[1])
    tg = tags.astype(np.int64)
    em_tag = np.take_along_axis(em, tg[:, :, None], axis=2)[:, :, 0]
    num = start_transitions.astype(np.float64)[tg[0]] + em_tag[0]
    trans_path = transitions.astype(np.float64)[tg[:-1], tg[1:]]
    num = num + np.sum((trans_path + em_tag[1:]) * maskf[1:], axis=0)
    seq_ends = mask.astype(np.int64).sum(axis=0) - 1
    last_tags = tg[seq_ends, b_idx]
    num = num + end_transitions.astype(np.float64)[last_tags]
    return num  # [B]


def _numpy_fallback(emissions, tags, mask, start_transitions, end_transitions, transitions):
    em = emissions.astype(np.float64)
    maskf = mask.astype(np.float64)
    Tn, Bn, Kn = em.shape
    num = _host_numerator(
        emissions, tags, mask, start_transitions, end_transitions, transitions
    )
    alpha = start_transitions.astype(np.float64)[None, :] + em[0]
    trans64 = transitions.astype(np.float64)
    for t in range(1, Tn):
        x = alpha[:, :, None] + trans64[None, :, :]
        m = x.max(axis=1)
        nxt = m + np.log(np.exp(x - m[:, None, :]).sum(axis=1)) + em[t]
        alpha = np.where(maskf[t][:, None] > 0, nxt, alpha)
    x = alpha + end_transitions.astype(np.float64)[None, :]
    m = x.max(axis=1)
    den = m + np.log(np.exp(x - m[:, None]).sum(axis=1))
    return np.float32(np.sum(num - den))


_PROGRAM_CACHE = {}


def kernel(emissions, tags, mask, start_transitions, end_transitions, transitions):
    emissions = np.asarray(emissions, np.float32)
    tags = np.asarray(tags, np.int32)
    mask = np.asarray(mask, np.int32)
    start_transitions = np.asarray(start_transitions, np.float32)
    end_transitions = np.asarray(end_transitions, np.float32)
    transitions = np.asarray(transitions, np.float32)

    if not np.all(mask == 1) or emissions.shape != (T, B, K):
        return _numpy_fallback(
            emissions, tags, mask, start_transitions, end_transitions, transitions
        )

    from concourse.bass_utils import run_bass_kernel_spmd

    if "nc" not in _PROGRAM_CACHE:
        _PROGRAM_CACHE["nc"] = _build_program()
    nc = _PROGRAM_CACHE["nc"]

    in_maps, warms = _host_prep(
        emissions, start_transitions, end_transitions, transitions
    )
    mexp_np = np.exp(transitions).astype(BF16)
    for m in in_maps:
        m["mexp"] = mexp_np

    res = run_bass_kernel_spmd(nc, in_maps, list(range(NCORES)))

    num = _host_numerator(
        emissions, tags, mask, start_transitions, end_transitions, transitions
    )
    total = float(num.sum())
    for c in range(NCORES):
        ev = res.results[c]["ev"].astype(np.float64)  # [K, NEVCOLS]
        den = np.full(BL, T * C0, np.float64)
        for p, (ci, k) in enumerate(_EV_BLOCKS):
            # final colsums of state cols 512k+v, v = 128h+m -> ev[m, 4p+h]
            cf = ev[:, 4 * p : 4 * p + 4].T.reshape(512)  # [512] state cols
            lncf = np.log(cf)
            for jj in range(8):  # 8 segs per 512-block
                j = 8 * k + jj
                sl = slice(jj * BL, (jj + 1) * BL)
                contrib = lncf[sl]
                if not (ci == 0 and j == 0):
                    contrib = contrib - np.log(warms[c][ci][j])
                den += contrib
        total -= den.sum()

    return np.float32(total)


# revision 32
# speedup vs baseline: 1.3435x; 1.3435x over previous
"""CRF loss (sum reduction) on 8 Trainium2 NeuronCores.

Strategy (data-parallel, batch sharded 8 ways, B_local=64 per core):
  * Denominator (log-partition): linear-space scaled forward algorithm.
    state[k,col]; step: state = (M^T state) * E_t with M = exp(transitions)
    as the stationary matmul lhsT and E_t = exp(em_t + bias - C0) computed
    ON HOST and streamed as bf16 (halves HBM traffic vs f32 em and removes
    the on-device exp pass entirely).
  * The serial T=512 scan is split into parallel-in-time segments, each
    warm-started one step early from a uniform vector (the transition
    matrix is a strong Hilbert-metric contraction, ~1e-2/step measured, so
    W=1 warmup leaves ~2e-4 nats of seam error). Segments are grouped into
    5 independent chains sized to balance engine load:
      - 2 "direct" chains (512 cols, 8 segs x 10 steps): DVE
        scalar_tensor_tensor straight from f32 PSUM (1x mode).
      - 2 "pair" chains (1024 cols, 16 segs x 9 steps): ScalarE evicts the
        two PSUM banks as one [K,1024] activation-copy to bf16 SBUF, then
        DVE multiplies all-bf16 at 2x mode.
      - 1 "solo" evict chain (512 cols, 8 segs x 8 steps).
    7 PSUM banks for the scan + 1 bank for events = 8.
  * No renormalization: per-column magnitudes stay O(1) by the -C0 bias;
    column sums are measured (ones-vector matmul -> one PSUM partition
    row) after the warmup row and after the last row; host takes logs.
  * Numerator (path score of the given tags) is exact and tiny
    (O(T*B) gathers): computed on host in f64.
"""

import sys
import numpy as np

for _p in ("/opt/trn_rl_repo",):
    if _p not in sys.path:
        sys.path.insert(0, _p)

import ml_dtypes

BF16 = ml_dtypes.bfloat16

T, B, K = 512, 512, 128
NCORES = 8
BL = B // NCORES            # 64 batch per core
C0 = 5.354                  # per-step log-scale compensation
W = 1                       # warmup rows per segment

# chain configs: (width_cols, TSEG, t0).  nseg = width // BL.
# kinds: direct = DVE STT straight from f32 PSUM (1x);
#        pair   = ScalarE [K,1024] eviction -> DVE bf16 TT (2x);
#        solo   = ScalarE [K,512] eviction -> DVE bf16 TT.
# coverage: 8*10 + 8*10 + 16*9 + 16*9 + 8*8 = 512 steps.
CHAINS = [
    dict(w=512,  tseg=10, t0=0,   kind="direct"),
    dict(w=512,  tseg=10, t0=80,  kind="direct"),
    dict(w=1024, tseg=9,  t0=160, kind="pair"),
    dict(w=1024, tseg=9,  t0=304, kind="pair"),
    dict(w=512,  tseg=8,  t0=448, kind="solo"),
]
for _c in CHAINS:
    _c["nseg"] = _c["w"] // BL
    _c["rows"] = _c["tseg"] + W
NROUND = max(c["rows"] for c in CHAINS)

# stream layout: round-major; within a round, chains in index order.
_CHUNK_START = []   # global col where round r starts
_SLOT_OFF = []      # per round: {chain: offset within round}
_ROUND_W = []
_off = 0
for _r in range(NROUND):
    _CHUNK_START.append(_off)
    offs = {}
    rw = 0
    for _ci, _c in enumerate(CHAINS):
        if _r < _c["rows"]:
            offs[_ci] = rw
            rw += _c["w"]
    _SLOT_OFF.append(offs)
    _ROUND_W.append(rw)
    _off += rw
NSTREAM = _off   # 36352

# final-event blocks: one per 512-col block of each chain; each block emits
# 4 transposed colsum matmuls ([K,128] stationary x ones -> [128,1]).
_EV_BLOCKS = []  # (chain_idx, block_idx)
for _ci, _c in enumerate(CHAINS):
    for _k in range(_c["w"] // 512):
        _EV_BLOCKS.append((_ci, _k))
NBLK = len(_EV_BLOCKS)          # 7
NEVCOLS = 4 * NBLK              # 28 psum cols, one per 128-state-col group


def _build_program():
    import concourse.bass as bass
    import concourse.tile as tile
    from concourse import mybir
    from contextlib import ExitStack

    # --- patch: walrus here rejects >1 sync-wait on the Tile final Drain ---
    from concourse.tile import ScopedClock

    def _patched_drain_and_barrier(self, tick_clock, wait_clock):
        nc = self.nc
        drain_inst = nc.sync.drain()
        wait_clock.add_sem_waits(
            drain_inst.ins, ScopedClock({None: tick_clock.global_clock})
        )
        si = drain_inst.ins.sync_info
        if si is not None and si.on_wait and len(si.on_wait) > 1:
            extra = list(si.on_wait[1:])
            del si.on_wait[1:]
            for w in extra:
                nop = nc.sync.nop()
                nop.ins.sync_info = mybir.SyncInfo(on_wait=[w], on_update=[])
        nc.all_engine_barrier()
        assert self.sems is not None
        popped = nc._tile_sem_poison_stack.pop()
        assert popped is self._sem_poison
        nc.clear_and_free_semaphores(list(self.sems.allocated().values()))
        nc.all_engine_barrier()

    tile.TileContext._drain_and_barrier = _patched_drain_and_barrier

    # --- patch 2: same walrus cap applies to every instruction type; spill
    # extra waits onto same-engine NOPs inserted just before. ---
    import bass_rust

    def _spill_excess_waits(nc_, cap=1):
        ctr = 0
        for f in nc_.m.functions:
            for bb in f.blocks:
                newlist = []
                for inst in bb.instructions:
                    si = getattr(inst, "sync_info", None)
                    if si is not None and si.on_wait and len(si.on_wait) > cap:
                        extra = list(si.on_wait[cap:])
                        del si.on_wait[cap:]
                        for w in extra:
                            ctr += 1
                            nop = bass_rust.InstNoOp(name=f"I-waitfix-{ctr}")
                            nop.engine = inst.engine
                            nop.sync_info = mybir.SyncInfo(on_wait=[w], on_update=[])
                            newlist.append(nop)
                    newlist.append(inst)
                bb.instructions[:] = newlist

    f32 = mybir.dt.float32
    bf16 = mybir.dt.bfloat16
    AF = mybir.ActivationFunctionType
    OP = mybir.AluOpType

    nc = bass.Bass()
    emt = nc.declare_dram_parameter("emt", [K, NSTREAM], bf16, isOutput=False)
    mexp = nc.declare_dram_parameter("mexp", [K, K], bf16, isOutput=False)
    ev_out = nc.declare_dram_parameter("ev", [K, NEVCOLS], f32, isOutput=True)

    with ExitStack() as ctx:
        tc = ctx.enter_context(tile.TileContext(nc))
        singles = ctx.enter_context(tc.tile_pool(name="singles", bufs=1))
        psum = ctx.enter_context(tc.tile_pool(name="psum", bufs=1, space="PSUM"))

        def tt_mult(out, in0, in1, eng=None):
            # true InstTensorTensor: hits DVE 2x_1p mode for all-bf16 SBUF
            # operands (scalar_tensor_tensor measures 1x on HW)
            v = eng if eng is not None else nc.vector
            return v.add_instruction(
                mybir.InstTensorTensor(
                    name=v.bass.get_next_instruction_name(),
                    op=OP.mult,
                    ins=[v.lower_ap(in0), v.lower_ap(in1)],
                    outs=[v.lower_ap(out)],
                )
            )

        # constants
        mexp_sb = singles.tile([K, K], bf16)
        nc.sync.dma_start(out=mexp_sb[:], in_=mexp[:])
        ones_k = singles.tile([K, 1], bf16)
        nc.vector.memset(ones_k[:], 1.0)

        # streamed E chunks: fine-grained early (fast scan start), coarse
        # later (per-chunk DMA bandwidth rises with size). All on the SP
        # HWDGE ring: the 16 SDMA engines are one shared pool, so a second
        # ring only interleaves packets and wrecks arrival order.
        groups = [[0], [1], [2], [3, 4], [5, 6], [7, 8], [9, 10]]
        Gtiles = []
        group_of_round = {}
        for gi, rounds in enumerate(groups):
            gw = sum(_ROUND_W[r] for r in rounds)
            gt = singles.tile([K, gw], bf16, name=f"Eg{gi}", tag=f"Eg{gi}")
            Gtiles.append(gt)
            for r in rounds:
                group_of_round[r] = gi
            lo = _CHUNK_START[rounds[0]]
            nc.sync.dma_start(out=gt[:], in_=emt[:, lo : lo + gw])

        def esl(r, ci, off=0, width=None):
            gi = group_of_round[r]
            base = _CHUNK_START[r] - _CHUNK_START[groups[gi][0]]
            lo = base + _SLOT_OFF[r][ci] + off
            w = CHAINS[ci]["w"] if width is None else width
            return Gtiles[gi][:, lo : lo + w]

        # states + evict buffers
        st = []
        ev_sb = []
        ps = []
        for ci, c in enumerate(CHAINS):
            s = singles.tile([K, c["w"]], bf16, name=f"st{ci}", tag=f"st{ci}")
            st.append(s)
            if c["kind"] != "direct":
                evb = singles.tile([K, c["w"]], bf16, name=f"evb{ci}", tag=f"evb{ci}")
                ev_sb.append(evb)
            else:
                ev_sb.append(None)
            psc = psum.tile([K, c["w"]], f32, name=f"ps{ci}", tag=f"ps{ci}")
            ps.append(psc)

        evt = psum.tile([K, NEVCOLS], f32, name="evt", tag="evt")
        evt_sb = singles.tile([K, NEVCOLS], f32, name="evt_sb")

        def emit_colsums(blocks):
            # transposed colsum: out[m,0] = sum_p st[p, base+m]
            for p, ci, k in blocks:
                for h in range(4):
                    base = 512 * k + 128 * h
                    nc.tensor.matmul(
                        evt[:, 4 * p + h : 4 * p + h + 1],
                        st[ci][:, base : base + 128],
                        ones_k[:],
                        start=True,
                        stop=True,
                    )

        # ---- the scan ----
        # row 0 is data-only: each segment warm-starts from E(t_w) directly
        # (M^T x ~ 1*colsum(x), so E(t_w) is already a one-step-warmed
        # direction); row 1's matmul reads the row-0 E slice as rhs.
        for r in range(1, NROUND):
            # longest dependency path first: pool, then pair, then direct
            prio = {"pool": 0, "pair": 1, "solo": 1, "direct": 2}
            order = sorted(range(len(CHAINS)), key=lambda ci: prio[CHAINS[ci]["kind"]])
            for ci in order:
                c = CHAINS[ci]
                if r >= c["rows"]:
                    continue
                Esl = esl(r, ci)
                # matmuls (one per 512-col bank); row 1 reads row-0 E as rhs
                for k in range(c["w"] // 512):
                    rhs = (
                        esl(0, ci, off=512 * k, width=512)
                        if r == 1
                        else st[ci][:, 512 * k : 512 * k + 512]
                    )
                    nc.tensor.matmul(
                        ps[ci][:, 512 * k : 512 * k + 512],
                        mexp_sb[:],
                        rhs,
                        start=True,
                        stop=True,
                    )
                if c["kind"] == "direct":
                    nc.vector.scalar_tensor_tensor(
                        out=st[ci][:],
                        in0=ps[ci][:],
                        scalar=1.0,
                        in1=Esl,
                        op0=OP.mult,
                        op1=OP.mult,
                    )
                else:
                    nc.scalar.activation(
                        ev_sb[ci][:], ps[ci][:], AF.Copy, bias=0.0, scale=1.0
                    )
                    tt_mult(st[ci][:], ev_sb[ci][:], Esl)
            if r == W:
                # exact reset of global segment 0 (chain 0, seg 0) to
                # a_0 = E_{t=0} (start bias folded in on host)
                nc.vector.tensor_copy(st[0][:, 0:BL], esl(W, 0, width=BL))
            # final colsum events for chains ending this round
            ending = [
                (p, ci, k)
                for p, (ci, k) in enumerate(_EV_BLOCKS)
                if CHAINS[ci]["rows"] == r + 1
            ]
            if ending:
                emit_colsums(ending)

        nc.vector.tensor_copy(evt_sb[:], evt[:])
        nc.sync.dma_start(out=ev_out[:], in_=evt_sb[:])

    _spill_excess_waits(nc)
    return nc


def _host_prep(emissions, start_transitions, end_transitions, transitions):
    """Per-core emt stream: E = exp(em + bias - C0) in bf16, round-major.

    Also returns the warm-start column sums (state after warmup row 0 is
    deterministically (M^T 1) * E, so ln of its colsum is host-computable).
    """
    in_maps = []
    warms = []
    biast = np.zeros((T, K), np.float32)
    biast[0] += start_transitions
    biast[-1] += end_transitions
    for c in range(NCORES):
        b0 = c * BL
        em_t = np.ascontiguousarray(
            emissions[:, b0 : b0 + BL, :].transpose(0, 2, 1)
        )  # [T, K, BL]
        Ebig = np.exp(em_t - C0 + biast[:, :, None]).astype(BF16)  # [T, K, BL]
        emt = np.empty((K, NSTREAM), BF16)
        for r in range(NROUND):
            for ci, ch in enumerate(CHAINS):
                if r >= ch["rows"]:
                    continue
                t_arr = np.clip(
                    ch["t0"] + np.arange(ch["nseg"]) * ch["tseg"] + (r - W), 0, T - 1
                )
                blk = Ebig[t_arr]  # [nseg, K, BL]
                lo = _CHUNK_START[r] + _SLOT_OFF[r][ci]
                emt[:, lo : lo + ch["w"]] = blk.transpose(1, 0, 2).reshape(
                    K, ch["w"]
                )
        in_maps.append({"emt": emt})
        wc = []
        for ch in CHAINS:
            t_w = np.clip(
                ch["t0"] + np.arange(ch["nseg"]) * ch["tseg"] - 1, 0, T - 1
            )
            # warm start is E(t_w) itself: warm colsum[j, b] = sum_k E
            wc.append(Ebig[t_w].astype(np.float64).sum(axis=1))
        warms.append(wc)
    return in_maps, warms


def _host_numerator(emissions, tags, mask, start_transitions, end_transitions,
                    transitions):
    em = emissions.astype(np.float64)
    maskf = mask.astype(np.float64)
    b_idx = np.arange(em.shape[1])
    tg = tags.astype(np.int64)
    em_tag = np.take_along_axis(em, tg[:, :, None], axis=2)[:, :, 0]
    num = start_transitions.astype(np.float64)[tg[0]] + em_tag[0]
    trans_path = transitions.astype(np.float64)[tg[:-1], tg[1:]]
    num = num + np.sum((trans_path + em_tag[1:]) * maskf[1:], axis=0)
    seq_ends = mask.astype(np.int64).sum(axis=0) - 1
    last_tags = tg[seq_ends, b_idx]
    num = num + end_transitions.astype(np.float64)[last_tags]
    return num  # [B]


def _numpy_fallback(emissions, tags, mask, start_transitions, end_transitions, transitions):
    em = emissions.astype(np.float64)
    maskf = mask.astype(np.float64)
    Tn, Bn, Kn = em.shape
    num = _host_numerator(
        emissions, tags, mask, start_transitions, end_transitions, transitions
    )
    alpha = start_transitions.astype(np.float64)[None, :] + em[0]
    trans64 = transitions.astype(np.float64)
    for t in range(1, Tn):
        x = alpha[:, :, None] + trans64[None, :, :]
        m = x.max(axis=1)
        nxt = m + np.log(np.exp(x - m[:, None, :]).sum(axis=1)) + em[t]
        alpha = np.where(maskf[t][:, None] > 0, nxt, alpha)
    x = alpha + end_transitions.astype(np.float64)[None, :]
    m = x.max(axis=1)
    den = m + np.log(np.exp(x - m[:, None]).sum(axis=1))
    return np.float32(np.sum(num - den))


_PROGRAM_CACHE = {}


def kernel(emissions, tags, mask, start_transitions, end_transitions, transitions):
    emissions = np.asarray(emissions, np.float32)
    tags = np.asarray(tags, np.int32)
    mask = np.asarray(mask, np.int32)
    start_transitions = np.asarray(start_transitions, np.float32)
    end_transitions = np.asarray(end_transitions, np.float32)
    transitions = np.asarray(transitions, np.float32)

    if not np.all(mask == 1) or emissions.shape != (T, B, K):
        return _numpy_fallback(
            emissions, tags, mask, start_transitions, end_transitions, transitions
        )

    from concourse.bass_utils import run_bass_kernel_spmd

    if "nc" not in _PROGRAM_CACHE:
        _PROGRAM_CACHE["nc"] = _build_program()
    nc = _PROGRAM_CACHE["nc"]

    in_maps, warms = _host_prep(
        emissions, start_transitions, end_transitions, transitions
    )
    mexp_np = np.exp(transitions).astype(BF16)
    for m in in_maps:
        m["mexp"] = mexp_np

    res = run_bass_kernel_spmd(nc, in_maps, list(range(NCORES)))

    num = _host_numerator(
        emissions, tags, mask, start_transitions, end_transitions, transitions
    )
    total = float(num.sum())
    for c in range(NCORES):
        ev = res.results[c]["ev"].astype(np.float64)  # [K, NEVCOLS]
        den = np.full(BL, T * C0, np.float64)
        for p, (ci, k) in enumerate(_EV_BLOCKS):
            # final colsums of state cols 512k+v, v = 128h+m -> ev[m, 4p+h]
            cf = ev[:, 4 * p : 4 * p + 4].T.reshape(512)  # [512] state cols
            lncf = np.log(cf)
            for jj in range(8):  # 8 segs per 512-block
                j = 8 * k + jj
                sl = slice(jj * BL, (jj + 1) * BL)
                contrib = lncf[sl]
                if not (ci == 0 and j == 0):
                    contrib = contrib - np.log(warms[c][ci][j])
                den += contrib
        total -= den.sum()

    return np.float32(total)


# revision 40
# speedup vs baseline: 1.3677x; 1.0180x over previous
"""CRF loss (sum reduction) on 8 Trainium2 NeuronCores.

Strategy (data-parallel, batch sharded 8 ways, B_local=64 per core):
  * Denominator (log-partition): linear-space scaled forward algorithm.
    state[k,col]; step: state = (M^T state) * E_t with M = exp(transitions)
    as the stationary matmul lhsT and E_t = exp(em_t + bias - C0) computed
    ON HOST and streamed as bf16 (halves HBM traffic vs f32 em and removes
    the on-device exp pass entirely).
  * The serial T=512 scan is split into parallel-in-time segments, each
    warm-started one step early from a uniform vector (the transition
    matrix is a strong Hilbert-metric contraction, ~1e-2/step measured, so
    W=1 warmup leaves ~2e-4 nats of seam error). Segments are grouped into
    5 independent chains sized to balance engine load:
      - 2 "direct" chains (512 cols, 8 segs x 10 steps): DVE
        scalar_tensor_tensor straight from f32 PSUM (1x mode).
      - 2 "pair" chains (1024 cols, 16 segs x 9 steps): ScalarE evicts the
        two PSUM banks as one [K,1024] activation-copy to bf16 SBUF, then
        DVE multiplies all-bf16 at 2x mode.
      - 1 "solo" evict chain (512 cols, 8 segs x 8 steps).
    7 PSUM banks for the scan + 1 bank for events = 8.
  * No renormalization: per-column magnitudes stay O(1) by the -C0 bias;
    column sums are measured (ones-vector matmul -> one PSUM partition
    row) after the warmup row and after the last row; host takes logs.
  * Numerator (path score of the given tags) is exact and tiny
    (O(T*B) gathers): computed on host in f64.
"""

import sys
import numpy as np

for _p in ("/opt/trn_rl_repo",):
    if _p not in sys.path:
        sys.path.insert(0, _p)

import ml_dtypes

BF16 = ml_dtypes.bfloat16

T, B, K = 512, 512, 128
NCORES = 8
BL = B // NCORES            # 64 batch per core
C0 = 5.354                  # per-step log-scale compensation
W = 1                       # warmup rows per segment

# chain configs: (width_cols, TSEG, t0).  nseg = width // BL.
# kinds: direct = DVE STT straight from f32 PSUM (1x);
#        pair   = ScalarE [K,1024] eviction -> DVE bf16 TT (2x);
#        solo   = ScalarE [K,512] eviction -> DVE bf16 TT.
# coverage: 8*10 + 8*10 + 16*9 + 16*9 + 8*8 = 512 steps.
CHAINS = [
    dict(w=512,  tseg=10, t0=0,   kind="direct"),
    dict(w=512,  tseg=10, t0=80,  kind="direct"),
    dict(w=1024, tseg=9,  t0=160, kind="pair"),
    dict(w=1024, tseg=9,  t0=304, kind="pair"),
    dict(w=512,  tseg=8,  t0=448, kind="solo"),
]
for _c in CHAINS:
    _c["nseg"] = _c["w"] // BL
    _c["rows"] = _c["tseg"] + W
NROUND = max(c["rows"] for c in CHAINS)

# stream layout: round-major; within a round, chains in index order.
_CHUNK_START = []   # global col where round r starts
_SLOT_OFF = []      # per round: {chain: offset within round}
_ROUND_W = []
_off = 0
for _r in range(NROUND):
    _CHUNK_START.append(_off)
    offs = {}
    rw = 0
    for _ci, _c in enumerate(CHAINS):
        if _r < _c["rows"]:
            offs[_ci] = rw
            rw += _c["w"]
    _SLOT_OFF.append(offs)
    _ROUND_W.append(rw)
    _off += rw
NSTREAM = _off   # 36352

# final-event blocks: one per 512-col block of each chain; each block emits
# 4 transposed colsum matmuls ([K,128] stationary x ones -> [128,1]).
_EV_BLOCKS = []  # (chain_idx, block_idx)
for _ci, _c in enumerate(CHAINS):
    for _k in range(_c["w"] // 512):
        _EV_BLOCKS.append((_ci, _k))
NBLK = len(_EV_BLOCKS)          # 7
NEVCOLS = 4 * NBLK              # 28 psum cols, one per 128-state-col group


def _build_program():
    import concourse.bass as bass
    import concourse.tile as tile
    from concourse import mybir
    from contextlib import ExitStack

    # --- patch: walrus here rejects >1 sync-wait on the Tile final Drain ---
    from concourse.tile import ScopedClock

    def _patched_drain_and_barrier(self, tick_clock, wait_clock):
        nc = self.nc
        drain_inst = nc.sync.drain()
        wait_clock.add_sem_waits(
            drain_inst.ins, ScopedClock({None: tick_clock.global_clock})
        )
        si = drain_inst.ins.sync_info
        if si is not None and si.on_wait and len(si.on_wait) > 1:
            extra = list(si.on_wait[1:])
            del si.on_wait[1:]
            for w in extra:
                nop = nc.sync.nop()
                nop.ins.sync_info = mybir.SyncInfo(on_wait=[w], on_update=[])
        nc.all_engine_barrier()
        assert self.sems is not None
        popped = nc._tile_sem_poison_stack.pop()
        assert popped is self._sem_poison
        nc.clear_and_free_semaphores(list(self.sems.allocated().values()))
        nc.all_engine_barrier()

    tile.TileContext._drain_and_barrier = _patched_drain_and_barrier

    # --- patch 2: same walrus cap applies to every instruction type; spill
    # extra waits onto same-engine NOPs inserted just before. ---
    import bass_rust

    def _spill_excess_waits(nc_, cap=1):
        ctr = 0
        for f in nc_.m.functions:
            for bb in f.blocks:
                newlist = []
                for inst in bb.instructions:
                    si = getattr(inst, "sync_info", None)
                    if si is not None and si.on_wait and len(si.on_wait) > cap:
                        extra = list(si.on_wait[cap:])
                        del si.on_wait[cap:]
                        for w in extra:
                            ctr += 1
                            nop = bass_rust.InstNoOp(name=f"I-waitfix-{ctr}")
                            nop.engine = inst.engine
                            nop.sync_info = mybir.SyncInfo(on_wait=[w], on_update=[])
                            newlist.append(nop)
                    newlist.append(inst)
                bb.instructions[:] = newlist

    def _dedup_ldweights(nc_):
        # bass emits an InstLdweights before every InstMatmult; the PE keeps
        # the stationary until the next load, so drop reloads of the same
        # weights (the scan reuses one [K,K] stationary throughout). Any sem
        # waits/updates on a dropped load are carried onto the next kept
        # instruction to preserve the sync protocol.
        for f in nc_.m.functions:
            for bb in f.blocks:
                newlist = []
                cur_sig = None
                pending = []
                for inst in bb.instructions:
                    tn = type(inst).__name__
                    if tn == "InstLdweights":
                        a = inst.ins[0]
                        sig = (a.memref, a.offset, str(a.ap), str(a.dtype))
                        if sig == cur_sig:
                            si = getattr(inst, "sync_info", None)
                            if si is not None and (si.on_wait or si.on_update):
                                pending.append(si)
                            continue
                        cur_sig = sig
                    if pending:
                        si = getattr(inst, "sync_info", None)
                        if si is None:
                            si = mybir.SyncInfo(on_wait=[], on_update=[])
                            inst.sync_info = si
                        for s in pending:
                            si.on_wait.extend(s.on_wait)
                            si.on_update.extend(s.on_update)
                        pending = []
                    newlist.append(inst)
                assert not pending
                bb.instructions[:] = newlist

    f32 = mybir.dt.float32
    bf16 = mybir.dt.bfloat16
    AF = mybir.ActivationFunctionType
    OP = mybir.AluOpType

    nc = bass.Bass()
    emt = nc.declare_dram_parameter("emt", [K, NSTREAM], bf16, isOutput=False)
    mexp = nc.declare_dram_parameter("mexp", [K, K], bf16, isOutput=False)
    ev_out = nc.declare_dram_parameter("ev", [K, NEVCOLS], f32, isOutput=True)

    with ExitStack() as ctx:
        tc = ctx.enter_context(tile.TileContext(nc))
        singles = ctx.enter_context(tc.tile_pool(name="singles", bufs=1))
        psum = ctx.enter_context(tc.tile_pool(name="psum", bufs=1, space="PSUM"))

        def tt_mult(out, in0, in1, eng=None):
            # true InstTensorTensor: hits DVE 2x_1p mode for all-bf16 SBUF
            # operands (scalar_tensor_tensor measures 1x on HW)
            v = eng if eng is not None else nc.vector
            return v.add_instruction(
                mybir.InstTensorTensor(
                    name=v.bass.get_next_instruction_name(),
                    op=OP.mult,
                    ins=[v.lower_ap(in0), v.lower_ap(in1)],
                    outs=[v.lower_ap(out)],
                )
            )

        # constants
        mexp_sb = singles.tile([K, K], bf16)
        nc.sync.dma_start(out=mexp_sb[:], in_=mexp[:])
        ones_k = singles.tile([K, 1], bf16)
        nc.vector.memset(ones_k[:], 1.0)

        # streamed E chunks, one per round, all on the SP HWDGE ring (FIFO
        # = arrival order; the 16 SDMA engines are one shared pool, so a
        # second ring only interleaves packets and wrecks arrival order,
        # and coarser chunks only delay per-round readiness).
        groups = [[r] for r in range(NROUND)]
        Gtiles = []
        group_of_round = {}
        for gi, rounds in enumerate(groups):
            gw = sum(_ROUND_W[r] for r in rounds)
            gt = singles.tile([K, gw], bf16, name=f"Eg{gi}", tag=f"Eg{gi}")
            Gtiles.append(gt)
            for r in rounds:
                group_of_round[r] = gi
            lo = _CHUNK_START[rounds[0]]
            nc.sync.dma_start(out=gt[:], in_=emt[:, lo : lo + gw])

        def esl(r, ci, off=0, width=None):
            gi = group_of_round[r]
            base = _CHUNK_START[r] - _CHUNK_START[groups[gi][0]]
            lo = base + _SLOT_OFF[r][ci] + off
            w = CHAINS[ci]["w"] if width is None else width
            return Gtiles[gi][:, lo : lo + w]

        # states + evict buffers
        st = []
        ev_sb = []
        ps = []
        for ci, c in enumerate(CHAINS):
            s = singles.tile([K, c["w"]], bf16, name=f"st{ci}", tag=f"st{ci}")
            st.append(s)
            if c["kind"] != "direct":
                evb = singles.tile([K, c["w"]], bf16, name=f"evb{ci}", tag=f"evb{ci}")
                ev_sb.append(evb)
            else:
                ev_sb.append(None)
            psc = psum.tile([K, c["w"]], f32, name=f"ps{ci}", tag=f"ps{ci}")
            ps.append(psc)

        evt = psum.tile([K, NEVCOLS], f32, name="evt", tag="evt")
        evt_sb = singles.tile([K, NEVCOLS], f32, name="evt_sb")

        def emit_colsums(blocks):
            # transposed colsum: out[m,0] = sum_p st[p, base+m]
            for p, ci, k in blocks:
                for h in range(4):
                    base = 512 * k + 128 * h
                    nc.tensor.matmul(
                        evt[:, 4 * p + h : 4 * p + h + 1],
                        st[ci][:, base : base + 128],
                        ones_k[:],
                        start=True,
                        stop=True,
                    )

        def mm_noload(out, rhs):
            # non-self-loading matmult: reuses the PE-resident stationary
            # (loaded once by a standalone ldweights), skipping the ~81ns
            # per-matmul LDWEIGHTS reload.
            mm = nc.tensor.matmul(out, mexp_sb[:], rhs, start=True, stop=True)
            mm.ins.ldweights = False
            return mm

        # ---- the scan ----
        # row 0 is data-only: each segment warm-starts from E(t_w) directly
        # (M^T x ~ 1*colsum(x), so E(t_w) is already a one-step-warmed
        # direction); row 1's matmul reads the row-0 E slice as rhs.
        nc.tensor.ldweights(mexp_sb[:])
        for r in range(1, NROUND):
            # longest dependency path first: pool, then pair, then direct
            prio = {"pool": 0, "pair": 1, "solo": 1, "direct": 2}
            order = sorted(range(len(CHAINS)), key=lambda ci: prio[CHAINS[ci]["kind"]])
            for ci in order:
                c = CHAINS[ci]
                if r >= c["rows"]:
                    continue
                Esl = esl(r, ci)
                # matmuls (one per 512-col bank); row 1 reads row-0 E as rhs
                for k in range(c["w"] // 512):
                    rhs = (
                        esl(0, ci, off=512 * k, width=512)
                        if r == 1
                        else st[ci][:, 512 * k : 512 * k + 512]
                    )
                    mm_noload(ps[ci][:, 512 * k : 512 * k + 512], rhs)
                if c["kind"] == "direct":
                    nc.vector.scalar_tensor_tensor(
                        out=st[ci][:],
                        in0=ps[ci][:],
                        scalar=1.0,
                        in1=Esl,
                        op0=OP.mult,
                        op1=OP.mult,
                    )
                else:
                    nc.scalar.activation(
                        ev_sb[ci][:], ps[ci][:], AF.Copy, bias=0.0, scale=1.0
                    )
                    tt_mult(st[ci][:], ev_sb[ci][:], Esl)
            if r == W:
                # exact reset of global segment 0 (chain 0, seg 0) to
                # a_0 = E_{t=0} (start bias folded in on host)
                nc.vector.tensor_copy(st[0][:, 0:BL], esl(W, 0, width=BL))
        # final colsum events: each chain's st is untouched after its last
        # round, so all events run after the scan (keeps the PE stream pure
        # mexp-stationary matmuls with zero reloads).
        emit_colsums([(p, ci, k) for p, (ci, k) in enumerate(_EV_BLOCKS)])

        nc.vector.tensor_copy(evt_sb[:], evt[:])
        nc.sync.dma_start(out=ev_out[:], in_=evt_sb[:])

    _dedup_ldweights(nc)
    _spill_excess_waits(nc)
    return nc


def _host_prep(emissions, start_transitions, end_transitions, transitions):
    """Per-core emt stream: E = exp(em + bias - C0) in bf16, round-major.

    Also returns the warm-start column sums (state after warmup row 0 is
    deterministically (M^T 1) * E, so ln of its colsum is host-computable).
    """
    in_maps = []
    warms = []
    biast = np.zeros((T, K), np.float32)
    biast[0] += start_transitions
    biast[-1] += end_transitions
    for c in range(NCORES):
        b0 = c * BL
        em_t = np.ascontiguousarray(
            emissions[:, b0 : b0 + BL, :].transpose(0, 2, 1)
        )  # [T, K, BL]
        Ebig = np.exp(em_t - C0 + biast[:, :, None]).astype(BF16)  # [T, K, BL]
        emt = np.empty((K, NSTREAM), BF16)
        for r in range(NROUND):
            for ci, ch in enumerate(CHAINS):
                if r >= ch["rows"]:
                    continue
                t_arr = np.clip(
                    ch["t0"] + np.arange(ch["nseg"]) * ch["tseg"] + (r - W), 0, T - 1
                )
                blk = Ebig[t_arr]  # [nseg, K, BL]
                lo = _CHUNK_START[r] + _SLOT_OFF[r][ci]
                emt[:, lo : lo + ch["w"]] = blk.transpose(1, 0, 2).reshape(
                    K, ch["w"]
                )
        in_maps.append({"emt": emt})
        wc = []
        for ch in CHAINS:
            t_w = np.clip(
                ch["t0"] + np.arange(ch["nseg"]) * ch["tseg"] - 1, 0, T - 1
            )
            # warm start is E(t_w) itself: warm colsum[j, b] = sum_k E
            wc.append(Ebig[t_w].astype(np.float64).sum(axis=1))
        warms.append(wc)
    return in_maps, warms


def _host_numerator(emissions, tags, mask, start_transitions, end_transitions,
                    transitions):
    em = emissions.astype(np.float64)
    maskf = mask.astype(np.float64)
    b_idx = np.arange(em.shape[1])
    tg = tags.astype(np.int64)
    em_tag = np.take_along_axis(em, tg[:, :, None], axis=2)[:, :, 0]
    num = start_transitions.astype(np.float64)[tg[0]] + em_tag[0]
    trans_path = transitions.astype(np.float64)[tg[:-1], tg[1:]]
    num = num + np.sum((trans_path + em_tag[1:]) * maskf[1:], axis=0)
    seq_ends = mask.astype(np.int64).sum(axis=0) - 1
    last_tags = tg[seq_ends, b_idx]
    num = num + end_transitions.astype(np.float64)[last_tags]
    return num  # [B]


def _numpy_fallback(emissions, tags, mask, start_transitions, end_transitions, transitions):
    em = emissions.astype(np.float64)
    maskf = mask.astype(np.float64)
    Tn, Bn, Kn = em.shape
    num = _host_numerator(
        emissions, tags, mask, start_transitions, end_transitions, transitions
    )
    alpha = start_transitions.astype(np.float64)[None, :] + em[0]
    trans64 = transitions.astype(np.float64)
    for t in range(1, Tn):
        x = alpha[:, :, None] + trans64[None, :, :]
        m = x.max(axis=1)
        nxt = m + np.log(np.exp(x - m[:, None, :]).sum(axis=1)) + em[t]
        alpha = np.where(maskf[t][:, None] > 0, nxt, alpha)
    x = alpha + end_transitions.astype(np.float64)[None, :]
    m = x.max(axis=1)
    den = m + np.log(np.exp(x - m[:, None]).sum(axis=1))
    return np.float32(np.sum(num - den))


_PROGRAM_CACHE = {}


def kernel(emissions, tags, mask, start_transitions, end_transitions, transitions):
    emissions = np.asarray(emissions, np.float32)
    tags = np.asarray(tags, np.int32)
    mask = np.asarray(mask, np.int32)
    start_transitions = np.asarray(start_transitions, np.float32)
    end_transitions = np.asarray(end_transitions, np.float32)
    transitions = np.asarray(transitions, np.float32)

    if not np.all(mask == 1) or emissions.shape != (T, B, K):
        return _numpy_fallback(
            emissions, tags, mask, start_transitions, end_transitions, transitions
        )

    from concourse.bass_utils import run_bass_kernel_spmd

    if "nc" not in _PROGRAM_CACHE:
        _PROGRAM_CACHE["nc"] = _build_program()
    nc = _PROGRAM_CACHE["nc"]

    in_maps, warms = _host_prep(
        emissions, start_transitions, end_transitions, transitions
    )
    mexp_np = np.exp(transitions).astype(BF16)
    for m in in_maps:
        m["mexp"] = mexp_np

    res = run_bass_kernel_spmd(nc, in_maps, list(range(NCORES)))

    num = _host_numerator(
        emissions, tags, mask, start_transitions, end_transitions, transitions
    )
    total = float(num.sum())
    for c in range(NCORES):
        ev = res.results[c]["ev"].astype(np.float64)  # [K, NEVCOLS]
        den = np.full(BL, T * C0, np.float64)
        for p, (ci, k) in enumerate(_EV_BLOCKS):
            # final colsums of state cols 512k+v, v = 128h+m -> ev[m, 4p+h]
            cf = ev[:, 4 * p : 4 * p + 4].T.reshape(512)  # [512] state cols
            lncf = np.log(cf)
            for jj in range(8):  # 8 segs per 512-block
                j = 8 * k + jj
                sl = slice(jj * BL, (jj + 1) * BL)
                contrib = lncf[sl]
                if not (ci == 0 and j == 0):
                    contrib = contrib - np.log(warms[c][ci][j])
                den += contrib
        total -= den.sum()

    return np.float32(total)


# revision 41
# speedup vs baseline: 1.4118x; 1.0322x over previous
"""CRF loss (sum reduction) on 8 Trainium2 NeuronCores.

Strategy (data-parallel, batch sharded 8 ways, B_local=64 per core):
  * Denominator (log-partition): linear-space scaled forward algorithm.
    state[k,col]; step: state = (M^T state) * E_t with M = exp(transitions)
    as the stationary matmul lhsT and E_t = exp(em_t + bias - C0) computed
    ON HOST and streamed as bf16 (halves HBM traffic vs f32 em and removes
    the on-device exp pass entirely).
  * The serial T=512 scan is split into parallel-in-time segments, each
    warm-started one step early from a uniform vector (the transition
    matrix is a strong Hilbert-metric contraction, ~1e-2/step measured, so
    W=1 warmup leaves ~2e-4 nats of seam error). Segments are grouped into
    5 independent chains sized to balance engine load:
      - 2 "direct" chains (512 cols, 8 segs x 10 steps): DVE
        scalar_tensor_tensor straight from f32 PSUM (1x mode).
      - 2 "pair" chains (1024 cols, 16 segs x 9 steps): ScalarE evicts the
        two PSUM banks as one [K,1024] activation-copy to bf16 SBUF, then
        DVE multiplies all-bf16 at 2x mode.
      - 1 "solo" evict chain (512 cols, 8 segs x 8 steps).
    7 PSUM banks for the scan + 1 bank for events = 8.
  * No renormalization: per-column magnitudes stay O(1) by the -C0 bias;
    column sums are measured (ones-vector matmul -> one PSUM partition
    row) after the warmup row and after the last row; host takes logs.
  * Numerator (path score of the given tags) is exact and tiny
    (O(T*B) gathers): computed on host in f64.
"""

import sys
import numpy as np

for _p in ("/opt/trn_rl_repo",):
    if _p not in sys.path:
        sys.path.insert(0, _p)

import ml_dtypes

BF16 = ml_dtypes.bfloat16

T, B, K = 512, 512, 128
NCORES = 8
BL = B // NCORES            # 64 batch per core
C0 = 5.354                  # per-step log-scale compensation
W = 1                       # warmup rows per segment

# chain configs: (width_cols, TSEG, t0).  nseg = width // BL.
# kinds: direct = DVE STT straight from f32 PSUM (1x);
#        pair   = ScalarE [K,1024] eviction -> DVE bf16 TT (2x);
#        solo   = ScalarE [K,512] eviction -> DVE bf16 TT.
# coverage: 8*10 + 8*10 + 16*9 + 16*9 + 8*8 = 512 steps.
CHAINS = [
    dict(w=512,  tseg=10, t0=0,   kind="direct"),
    dict(w=512,  tseg=10, t0=80,  kind="direct"),
    dict(w=1024, tseg=9,  t0=160, kind="pair"),
    dict(w=1024, tseg=9,  t0=304, kind="pair"),
    dict(w=512,  tseg=8,  t0=448, kind="solo"),
]
for _c in CHAINS:
    _c["nseg"] = _c["w"] // BL
    _c["rows"] = _c["tseg"] + W
NROUND = max(c["rows"] for c in CHAINS)

# stream layout: round-major; within a round, chains in index order.
_CHUNK_START = []   # global col where round r starts
_SLOT_OFF = []      # per round: {chain: offset within round}
_ROUND_W = []
_off = 0
for _r in range(NROUND):
    _CHUNK_START.append(_off)
    offs = {}
    rw = 0
    for _ci, _c in enumerate(CHAINS):
        if _r < _c["rows"]:
            offs[_ci] = rw
            rw += _c["w"]
    _SLOT_OFF.append(offs)
    _ROUND_W.append(rw)
    _off += rw
NSTREAM = _off   # 36352

# final-event blocks: one per 512-col block of each chain; each block emits
# 4 transposed colsum matmuls ([K,128] stationary x ones -> [128,1]).
_EV_BLOCKS = []  # (chain_idx, block_idx)
for _ci, _c in enumerate(CHAINS):
    for _k in range(_c["w"] // 512):
        _EV_BLOCKS.append((_ci, _k))
NBLK = len(_EV_BLOCKS)          # 7
NEVCOLS = 4 * NBLK              # 28 psum cols, one per 128-state-col group


def _build_program():
    import concourse.bass as bass
    import concourse.tile as tile
    from concourse import mybir
    from contextlib import ExitStack

    # --- patch: walrus here rejects >1 sync-wait on the Tile final Drain ---
    from concourse.tile import ScopedClock

    def _patched_drain_and_barrier(self, tick_clock, wait_clock):
        nc = self.nc
        drain_inst = nc.sync.drain()
        wait_clock.add_sem_waits(
            drain_inst.ins, ScopedClock({None: tick_clock.global_clock})
        )
        si = drain_inst.ins.sync_info
        if si is not None and si.on_wait and len(si.on_wait) > 1:
            extra = list(si.on_wait[1:])
            del si.on_wait[1:]
            for w in extra:
                nop = nc.sync.nop()
                nop.ins.sync_info = mybir.SyncInfo(on_wait=[w], on_update=[])
        nc.all_engine_barrier()
        assert self.sems is not None
        popped = nc._tile_sem_poison_stack.pop()
        assert popped is self._sem_poison
        nc.clear_and_free_semaphores(list(self.sems.allocated().values()))
        nc.all_engine_barrier()

    tile.TileContext._drain_and_barrier = _patched_drain_and_barrier

    # --- patch 2: same walrus cap applies to every instruction type; spill
    # extra waits onto same-engine NOPs inserted just before. ---
    import bass_rust

    def _spill_excess_waits(nc_, cap=1):
        ctr = 0
        for f in nc_.m.functions:
            for bb in f.blocks:
                newlist = []
                for inst in bb.instructions:
                    si = getattr(inst, "sync_info", None)
                    if si is not None and si.on_wait and len(si.on_wait) > cap:
                        extra = list(si.on_wait[cap:])
                        del si.on_wait[cap:]
                        for w in extra:
                            ctr += 1
                            nop = bass_rust.InstNoOp(name=f"I-waitfix-{ctr}")
                            nop.engine = inst.engine
                            nop.sync_info = mybir.SyncInfo(on_wait=[w], on_update=[])
                            newlist.append(nop)
                    newlist.append(inst)
                bb.instructions[:] = newlist

    def _dedup_ldweights(nc_):
        # bass emits an InstLdweights before every InstMatmult; the PE keeps
        # the stationary until the next load, so drop reloads of the same
        # weights (the scan reuses one [K,K] stationary throughout). Any sem
        # waits/updates on a dropped load are carried onto the next kept
        # instruction to preserve the sync protocol.
        for f in nc_.m.functions:
            for bb in f.blocks:
                newlist = []
                cur_sig = None
                pending = []
                for inst in bb.instructions:
                    tn = type(inst).__name__
                    if tn == "InstLdweights":
                        a = inst.ins[0]
                        sig = (a.memref, a.offset, str(a.ap), str(a.dtype))
                        if sig == cur_sig:
                            si = getattr(inst, "sync_info", None)
                            if si is not None and (si.on_wait or si.on_update):
                                pending.append(si)
                            continue
                        cur_sig = sig
                    if pending:
                        si = getattr(inst, "sync_info", None)
                        if si is None:
                            si = mybir.SyncInfo(on_wait=[], on_update=[])
                            inst.sync_info = si
                        for s in pending:
                            si.on_wait.extend(s.on_wait)
                            si.on_update.extend(s.on_update)
                        pending = []
                    newlist.append(inst)
                assert not pending
                bb.instructions[:] = newlist

    f32 = mybir.dt.float32
    bf16 = mybir.dt.bfloat16
    AF = mybir.ActivationFunctionType
    OP = mybir.AluOpType

    nc = bass.Bass()
    emt = nc.declare_dram_parameter("emt", [K, NSTREAM], bf16, isOutput=False)
    mexp = nc.declare_dram_parameter("mexp", [K, K], bf16, isOutput=False)
    ev_out = nc.declare_dram_parameter("ev", [K, NEVCOLS], f32, isOutput=True)

    with ExitStack() as ctx:
        tc = ctx.enter_context(tile.TileContext(nc))
        singles = ctx.enter_context(tc.tile_pool(name="singles", bufs=1))
        psum = ctx.enter_context(tc.tile_pool(name="psum", bufs=1, space="PSUM"))

        def tt_mult(out, in0, in1, eng=None):
            # true InstTensorTensor: hits DVE 2x_1p mode for all-bf16 SBUF
            # operands (scalar_tensor_tensor measures 1x on HW)
            v = eng if eng is not None else nc.vector
            return v.add_instruction(
                mybir.InstTensorTensor(
                    name=v.bass.get_next_instruction_name(),
                    op=OP.mult,
                    ins=[v.lower_ap(in0), v.lower_ap(in1)],
                    outs=[v.lower_ap(out)],
                )
            )

        # constants
        mexp_sb = singles.tile([K, K], bf16)
        nc.sync.dma_start(out=mexp_sb[:], in_=mexp[:])
        ones_k = singles.tile([K, 1], bf16)
        nc.vector.memset(ones_k[:], 1.0)

        # streamed E chunks, one per round, all on the SP HWDGE ring (FIFO
        # = arrival order; the 16 SDMA engines are one shared pool, so a
        # second ring only interleaves packets and wrecks arrival order,
        # and coarser chunks only delay per-round readiness).
        groups = [[r] for r in range(NROUND)]
        Gtiles = []
        group_of_round = {}
        for gi, rounds in enumerate(groups):
            gw = sum(_ROUND_W[r] for r in rounds)
            gt = singles.tile([K, gw], bf16, name=f"Eg{gi}", tag=f"Eg{gi}")
            Gtiles.append(gt)
            for r in rounds:
                group_of_round[r] = gi
            lo = _CHUNK_START[rounds[0]]
            nc.sync.dma_start(out=gt[:], in_=emt[:, lo : lo + gw])

        def esl(r, ci, off=0, width=None):
            gi = group_of_round[r]
            base = _CHUNK_START[r] - _CHUNK_START[groups[gi][0]]
            lo = base + _SLOT_OFF[r][ci] + off
            w = CHAINS[ci]["w"] if width is None else width
            return Gtiles[gi][:, lo : lo + w]

        # states + evict buffers
        st = []
        ev_sb = []
        ps = []
        for ci, c in enumerate(CHAINS):
            s = singles.tile([K, c["w"]], bf16, name=f"st{ci}", tag=f"st{ci}")
            st.append(s)
            if c["kind"] != "direct":
                evb = singles.tile([K, c["w"]], bf16, name=f"evb{ci}", tag=f"evb{ci}")
                ev_sb.append(evb)
            else:
                ev_sb.append(None)
            psc = psum.tile([K, c["w"]], f32, name=f"ps{ci}", tag=f"ps{ci}")
            ps.append(psc)

        evt = psum.tile([K, NEVCOLS], f32, name="evt", tag="evt")
        evt_sb = singles.tile([K, NEVCOLS], f32, name="evt_sb")

        def emit_colsums(blocks):
            # transposed colsum: out[m,0] = sum_p st[p, base+m]
            for p, ci, k in blocks:
                for h in range(4):
                    base = 512 * k + 128 * h
                    nc.tensor.matmul(
                        evt[:, 4 * p + h : 4 * p + h + 1],
                        st[ci][:, base : base + 128],
                        ones_k[:],
                        start=True,
                        stop=True,
                    )

        def mm_noload(out, rhs):
            # non-self-loading matmult: reuses the PE-resident stationary
            # (loaded once by a standalone ldweights), skipping the ~81ns
            # per-matmul LDWEIGHTS reload.
            mm = nc.tensor.matmul(out, mexp_sb[:], rhs, start=True, stop=True)
            mm.ins.ldweights = False
            return mm

        # ---- the scan ----
        # row 0 is data-only: each segment warm-starts from E(t_w) directly
        # (M^T x ~ 1*colsum(x), so E(t_w) is already a one-step-warmed
        # direction); row 1's matmul reads the row-0 E slice as rhs.
        nc.tensor.ldweights(mexp_sb[:])
        for r in range(1, NROUND):
            # longest dependency path first: pool, then pair, then direct
            prio = {"pool": 0, "pair": 1, "solo": 1, "direct": 2}
            order = sorted(range(len(CHAINS)), key=lambda ci: prio[CHAINS[ci]["kind"]])
            for ci in order:
                c = CHAINS[ci]
                if r >= c["rows"]:
                    continue
                Esl = esl(r, ci)
                # matmuls (one per 512-col bank); row 1 reads row-0 E as rhs
                for k in range(c["w"] // 512):
                    rhs = (
                        esl(0, ci, off=512 * k, width=512)
                        if r == 1
                        else st[ci][:, 512 * k : 512 * k + 512]
                    )
                    mm_noload(ps[ci][:, 512 * k : 512 * k + 512], rhs)
                if c["kind"] == "direct":
                    nc.vector.scalar_tensor_tensor(
                        out=st[ci][:],
                        in0=ps[ci][:],
                        scalar=1.0,
                        in1=Esl,
                        op0=OP.mult,
                        op1=OP.mult,
                    )
                else:
                    nc.scalar.activation(
                        ev_sb[ci][:], ps[ci][:], AF.Copy, bias=0.0, scale=1.0
                    )
                    tt_mult(st[ci][:], ev_sb[ci][:], Esl)
            if r == W:
                # exact reset of global segment 0 (chain 0, seg 0) to
                # a_0 = E_{t=0} (start bias folded in on host)
                nc.vector.tensor_copy(st[0][:, 0:BL], esl(W, 0, width=BL))
            # final colsum events for chains ending this round (the ldweights
            # dedup pass keeps the stationary switches cheap)
            ending = [
                (p, ci, k)
                for p, (ci, k) in enumerate(_EV_BLOCKS)
                if CHAINS[ci]["rows"] == r + 1
            ]
            if ending:
                emit_colsums(ending)

        nc.vector.tensor_copy(evt_sb[:], evt[:])
        nc.sync.dma_start(out=ev_out[:], in_=evt_sb[:])

    _dedup_ldweights(nc)
    _spill_excess_waits(nc)
    return nc


def _host_prep(emissions, start_transitions, end_transitions, transitions):
    """Per-core emt stream: E = exp(em + bias - C0) in bf16, round-major.

    Also returns the warm-start column sums (state after warmup row 0 is
    deterministically (M^T 1) * E, so ln of its colsum is host-computable).
    """
    in_maps = []
    warms = []
    biast = np.zeros((T, K), np.float32)
    biast[0] += start_transitions
    biast[-1] += end_transitions
    for c in range(NCORES):
        b0 = c * BL
        em_t = np.ascontiguousarray(
            emissions[:, b0 : b0 + BL, :].transpose(0, 2, 1)
        )  # [T, K, BL]
        Ebig = np.exp(em_t - C0 + biast[:, :, None]).astype(BF16)  # [T, K, BL]
        emt = np.empty((K, NSTREAM), BF16)
        for r in range(NROUND):
            for ci, ch in enumerate(CHAINS):
                if r >= ch["rows"]:
                    continue
                t_arr = np.clip(
                    ch["t0"] + np.arange(ch["nseg"]) * ch["tseg"] + (r - W), 0, T - 1
                )
                blk = Ebig[t_arr]  # [nseg, K, BL]
                lo = _CHUNK_START[r] + _SLOT_OFF[r][ci]
                emt[:, lo : lo + ch["w"]] = blk.transpose(1, 0, 2).reshape(
                    K, ch["w"]
                )
        in_maps.append({"emt": emt})
        wc = []
        for ch in CHAINS:
            t_w = np.clip(
                ch["t0"] + np.arange(ch["nseg"]) * ch["tseg"] - 1, 0, T - 1
            )
            # warm start is E(t_w) itself: warm colsum[j, b] = sum_k E
            wc.append(Ebig[t_w].astype(np.float64).sum(axis=1))
        warms.append(wc)
    return in_maps, warms


def _host_numerator(emissions, tags, mask, start_transitions, end_transitions,
                    transitions):
    em = emissions.astype(np.float64)
    maskf = mask.astype(np.float64)
    b_idx = np.arange(em.shape[1])
    tg = tags.astype(np.int64)
    em_tag = np.take_along_axis(em, tg[:, :, None], axis=2)[:, :, 0]
    num = start_transitions.astype(np.float64)[tg[0]] + em_tag[0]
    trans_path = transitions.astype(np.float64)[tg[:-1], tg[1:]]
    num = num + np.sum((trans_path + em_tag[1:]) * maskf[1:], axis=0)
    seq_ends = mask.astype(np.int64).sum(axis=0) - 1
    last_tags = tg[seq_ends, b_idx]
    num = num + end_transitions.astype(np.float64)[last_tags]
    return num  # [B]


def _numpy_fallback(emissions, tags, mask, start_transitions, end_transitions, transitions):
    em = emissions.astype(np.float64)
    maskf = mask.astype(np.float64)
    Tn, Bn, Kn = em.shape
    num = _host_numerator(
        emissions, tags, mask, start_transitions, end_transitions, transitions
    )
    alpha = start_transitions.astype(np.float64)[None, :] + em[0]
    trans64 = transitions.astype(np.float64)
    for t in range(1, Tn):
        x = alpha[:, :, None] + trans64[None, :, :]
        m = x.max(axis=1)
        nxt = m + np.log(np.exp(x - m[:, None, :]).sum(axis=1)) + em[t]
        alpha = np.where(maskf[t][:, None] > 0, nxt, alpha)
    x = alpha + end_transitions.astype(np.float64)[None, :]
    m = x.max(axis=1)
    den = m + np.log(np.exp(x - m[:, None]).sum(axis=1))
    return np.float32(np.sum(num - den))


_PROGRAM_CACHE = {}


def kernel(emissions, tags, mask, start_transitions, end_transitions, transitions):
    emissions = np.asarray(emissions, np.float32)
    tags = np.asarray(tags, np.int32)
    mask = np.asarray(mask, np.int32)
    start_transitions = np.asarray(start_transitions, np.float32)
    end_transitions = np.asarray(end_transitions, np.float32)
    transitions = np.asarray(transitions, np.float32)

    if not np.all(mask == 1) or emissions.shape != (T, B, K):
        return _numpy_fallback(
            emissions, tags, mask, start_transitions, end_transitions, transitions
        )

    from concourse.bass_utils import run_bass_kernel_spmd

    if "nc" not in _PROGRAM_CACHE:
        _PROGRAM_CACHE["nc"] = _build_program()
    nc = _PROGRAM_CACHE["nc"]

    in_maps, warms = _host_prep(
        emissions, start_transitions, end_transitions, transitions
    )
    mexp_np = np.exp(transitions).astype(BF16)
    for m in in_maps:
        m["mexp"] = mexp_np

    res = run_bass_kernel_spmd(nc, in_maps, list(range(NCORES)))

    num = _host_numerator(
        emissions, tags, mask, start_transitions, end_transitions, transitions
    )
    total = float(num.sum())
    for c in range(NCORES):
        ev = res.results[c]["ev"].astype(np.float64)  # [K, NEVCOLS]
        den = np.full(BL, T * C0, np.float64)
        for p, (ci, k) in enumerate(_EV_BLOCKS):
            # final colsums of state cols 512k+v, v = 128h+m -> ev[m, 4p+h]
            cf = ev[:, 4 * p : 4 * p + 4].T.reshape(512)  # [512] state cols
            lncf = np.log(cf)
            for jj in range(8):  # 8 segs per 512-block
                j = 8 * k + jj
                sl = slice(jj * BL, (jj + 1) * BL)
                contrib = lncf[sl]
                if not (ci == 0 and j == 0):
                    contrib = contrib - np.log(warms[c][ci][j])
                den += contrib
        total -= den.sum()

    return np.float32(total)
